# revision 19
# baseline (speedup 1.0000x reference)
"""Trainium2 Bass kernel for nn_AdjEnsemble (gnn_message_passing).

Math: softmax rows of adj sum to 1, so adj_norm @ (sv - c_k) = adj_norm@sv - c_k.
With E = exp(-adj) (no max-subtraction needed: adj ~ N(0,1)) and
R = rowsum(E), the whole module collapses to

    t        = (E @ sv) / R                      # [N, D]
    features = mean_k lrelu(t - c_k)
             = 0.2475 * sum_k relu(t - c_k) + 0.01*(t - mean_k c_k)
    out      = relu(features @ fc_w.T + fc_b)    # [N, OUT]

Sharding: adj rows split across 8 cores ([1024, 8192] each); everything
else replicated. No collectives: each core's output rows stay local.

Per-core dataflow: DMA adj tiles (natural layout, f32) -> PE transpose
128x128 blocks into PSUM -> ACT exp(-x) PSUM->SBUF (bf16) -> PE matmul
accumulate S^T[65, m] = svaug^T @ E^T over n-chunks (svaug has a ones
column so row 64 of S^T is the softmax denominator R) -> small epilogue
on DVE -> fc matmul (bias folded in as an extra contraction row) -> relu
-> DMA out.
"""

import numpy as np
import ml_dtypes

_BF16 = ml_dtypes.bfloat16

N, D, K, OUT = 8192, 64, 4, 256
N_CORES = 8
M_SH = N // N_CORES          # 1024 adj rows per core
DA = D + 1                   # 65: sv columns + ones column
MSUP = 512                   # max m rows accumulated per PSUM accumulator
SUPS = [512, 384, 128]       # uneven m_super widths: tiny last epilogue tail
NBLK = 2048                  # n columns per DMA'd adj tile
NCH = 128                    # n contraction chunk (PE partition dim)
LRELU_SLOPE = 0.01
RELU_COEF = (1.0 - LRELU_SLOPE) / 4.0       # 0.2475
G = LRELU_SLOPE / RELU_COEF                 # linear-term coefficient inside feat'

_GRAPH_CACHE = {}


def _build_graph():
    if "nc" in _GRAPH_CACHE:
        return _GRAPH_CACHE["nc"]

    import concourse.tile as tile
    from concourse import bacc, mybir

    f32 = mybir.dt.float32
    f32r = mybir.dt.float32r
    bf16 = mybir.dt.bfloat16
    Act = mybir.ActivationFunctionType
    Alu = mybir.AluOpType

    nc = bacc.Bacc("TRN2", target_bir_lowering=False, debug=False,
                   num_devices=N_CORES)

    adj_ext = nc.declare_dram_parameter("adj", [M_SH, N], f32r, isOutput=False)
    svp_ext = nc.declare_dram_parameter("svp", [128, (N // NCH) * DA], bf16,
                                        isOutput=False)
    id_ext = nc.declare_dram_parameter("ident", [128, 128], f32r, isOutput=False)
    wt_ext = nc.declare_dram_parameter("wt", [DA, OUT + 1], bf16, isOutput=False)
    epi_ext = nc.declare_dram_parameter("epi", [D, 8], f32, isOutput=False)
    out_ext = nc.declare_dram_parameter("out", [M_SH, OUT], f32, isOutput=True)

    n_msup = M_SH // MSUP            # 2
    n_nblk = N // NBLK               # 4
    n_nch = NBLK // NCH              # 16
    n_sub = MSUP // 128              # 4

    with tile.TileContext(nc) as tc:
        with (
            tc.tile_pool(name="const", bufs=1) as const,
            tc.tile_pool(name="a", bufs=2 * n_sub) as apool,
            tc.tile_pool(name="et", bufs=3) as etpool,
            tc.tile_pool(name="epi", bufs=2) as epool,
            tc.tile_pool(name="osb", bufs=2) as outp,
            tc.tile_pool(name="stage", bufs=2, space="PSUM") as stpool,
            tc.tile_pool(name="acc", bufs=2, space="PSUM") as accpool,
            tc.tile_pool(name="ops", bufs=2, space="PSUM") as opool,
        ):
            svt = const.tile([128, (N // NCH) * DA], bf16)
            nc.sync.dma_start(svt[:], svp_ext[:])
            idt = const.tile([128, 128], f32r)
            nc.sync.dma_start(idt[:], id_ext[:])
            wtt = const.tile([DA, OUT + 1], bf16)
            nc.sync.dma_start(wtt[:], wt_ext[:])
            epit = const.tile([D, 8], f32)
            nc.sync.dma_start(epit[:], epi_ext[:])
            onest = const.tile([1, D], f32)
            nc.vector.memset(onest[:], 1.0)

            def emit_main(su, acc, nb_range, dma_eng=None):
                m0 = sum(SUPS[:su])
                w = SUPS[su]
                nsub = w // 128
                for nb in nb_range:
                    ats = []
                    for s in range(nsub):
                        at = apool.tile([128, NBLK], f32r, tag="a")
                        eng = dma_eng or nc.gpsimd
                        eng.dma_start(
                            at[:],
                            adj_ext[m0 + s * 128:m0 + (s + 1) * 128,
                                    nb * NBLK:(nb + 1) * NBLK])
                        ats.append(at)
                    for nch2 in range(n_nch // 2):
                        stage = stpool.tile([128, 2 * w], f32r, tag="stage")
                        for j in range(2):
                            nch = nch2 * 2 + j
                            for s in range(nsub):
                                nc.tensor.transpose(
                                    stage[:, j * w + s * 128:
                                          j * w + (s + 1) * 128],
                                    ats[s][:, nch * NCH:(nch + 1) * NCH],
                                    idt[:])
                        et = etpool.tile([128, 2 * w], bf16, tag="et")
                        nc.scalar.activation(et[:], stage[:], Act.Exp,
                                             scale=-1.0)
                        for j in range(2):
                            chunk = nb * n_nch + nch2 * 2 + j
                            nc.tensor.matmul(
                                acc[:, 0:w],
                                svt[:, chunk * DA:(chunk + 1) * DA],
                                et[:, j * w:(j + 1) * w],
                                start=(chunk == 0),
                                stop=(chunk == (N // NCH) - 1))

            def emit_epilogue(su, acc):
                # acc[0:64] = S^T rows, acc[64] = R (softmax denominator).
                # Division by R is deferred: G = sum_k relu(S - R c_k)
                # + g (S - R cbar); the fc matmul passes R through an extra
                # output column, and the final relu+divide happens in natural
                # layout where 1/R is a per-partition scalar.
                m0 = sum(SUPS[:su])
                w = SUPS[su]
                rsb = epool.tile([1, MSUP], f32, tag="rsb", name=f"rsb{su}")
                nc.vector.tensor_copy(rsb[0:1, 0:w], acc[D:DA, 0:w])
                rb_ps = opool.tile([D, MSUP], f32, tag="ops", name=f"rbp{su}")
                nc.tensor.matmul(rb_ps[:, 0:w], onest[:], rsb[0:1, 0:w],
                                 start=True, stop=True)
                rbs = epool.tile([D, MSUP], f32, tag="rbs", name=f"rbs{su}")
                nc.vector.tensor_copy(rbs[:, 0:w], rb_ps[:, 0:w])

                rk = []
                for k in range(K):
                    v = epool.tile([D, MSUP], f32, tag=f"v{k}",
                                   name=f"v{k}_{su}")
                    nc.vector.tensor_scalar_mul(v[:, 0:w], rbs[:, 0:w],
                                                epit[:, k:k + 1])
                    u = epool.tile([D, MSUP], f32, tag=f"u{k}",
                                   name=f"u{k}_{su}")
                    nc.vector.tensor_tensor(u[:, 0:w], acc[0:D, 0:w],
                                            v[:, 0:w], Alu.subtract)
                    r = epool.tile([D, MSUP], f32, tag=f"r{k}",
                                   name=f"r{k}_{su}")
                    nc.scalar.activation(r[:, 0:w], u[:, 0:w], Act.Relu)
                    rk.append(r)
                vb = epool.tile([D, MSUP], f32, tag="vb", name=f"vb{su}")
                nc.vector.tensor_scalar_mul(vb[:, 0:w], rbs[:, 0:w],
                                            epit[:, 4:5])
                qs = epool.tile([D, MSUP], f32, tag="qs", name=f"qs{su}")
                nc.vector.tensor_tensor(qs[:, 0:w], acc[0:D, 0:w],
                                        vb[:, 0:w], Alu.subtract)
                s01 = epool.tile([D, MSUP], f32, tag="s01", name=f"s01_{su}")
                nc.vector.tensor_tensor(s01[:, 0:w], rk[0][:, 0:w],
                                        rk[1][:, 0:w], Alu.add)
                s23 = epool.tile([D, MSUP], f32, tag="s23", name=f"s23_{su}")
                nc.vector.tensor_tensor(s23[:, 0:w], rk[2][:, 0:w],
                                        rk[3][:, 0:w], Alu.add)
                s03 = epool.tile([D, MSUP], f32, tag="s03", name=f"s03_{su}")
                nc.vector.tensor_tensor(s03[:, 0:w], s01[:, 0:w],
                                        s23[:, 0:w], Alu.add)
                qg = epool.tile([D, MSUP], f32, tag="qg", name=f"qg{su}")
                nc.vector.tensor_scalar(qg[:, 0:w], qs[:, 0:w], G, 0.0,
                                        Alu.mult, Alu.bypass)
                feat = epool.tile([DA, MSUP], bf16, tag="feat",
                                  name=f"feat{su}")
                nc.vector.tensor_copy(feat[D:DA, 0:w], rsb[0:1, 0:w])
                nc.vector.tensor_tensor(feat[0:D, 0:w], s03[:, 0:w],
                                        qg[:, 0:w], Alu.add)

                for mc in range(w // 128):
                    ops = opool.tile([128, OUT + 1], f32, tag="ops",
                                     name=f"ops{su}_{mc}")
                    nc.tensor.matmul(ops[:],
                                     feat[:, mc * 128:(mc + 1) * 128],
                                     wtt[:], start=True, stop=True)
                    rinv = outp.tile([128, 1], f32, tag="rinv",
                                     name=f"rinv{su}_{mc}")
                    nc.vector.reciprocal(rinv[:], ops[:, OUT:OUT + 1])
                    osb = outp.tile([128, OUT], f32, tag="osb",
                                    name=f"osb{su}_{mc}")
                    nc.vector.tensor_scalar(osb[:], ops[:, 0:OUT], 0.0,
                                            rinv[:], Alu.max, Alu.mult)
                    nc.sync.dma_start(
                        out_ext[m0 + mc * 128:m0 + (mc + 1) * 128, :],
                        osb[:])

            # PE warmup: dummy matmuls on the identity engage the HAM
            # clock-gate and bridge the initial DMA fill.
            warm = stpool.tile([128, 128], f32, tag="stage", name="warm")
            for _ in range(40):
                nc.tensor.matmul(warm[:], idt[:], idt[:],
                                 start=True, stop=True)

            accs = [accpool.tile([DA, MSUP], f32, tag="acc", name=f"acc{i}")
                    for i in range(len(SUPS))]
            emit_main(0, accs[0], range(0, 1), dma_eng=nc.sync)
            emit_main(0, accs[0], range(1, n_nblk))
            emit_main(1, accs[1], range(0, 2))
            emit_epilogue(0, accs[0])
            emit_main(1, accs[1], range(2, n_nblk))
            emit_main(2, accs[2], range(0, 2))
            emit_epilogue(1, accs[1])
            emit_main(2, accs[2], range(2, n_nblk))
            emit_epilogue(2, accs[2])

    nc.compile()
    _GRAPH_CACHE["nc"] = nc
    return nc


def _prep_in_maps(semantic_vec, adj, field_centers, fc_w, fc_b):
    svaug = np.concatenate(
        [semantic_vec.astype(np.float32),
         np.ones((N, 1), np.float32)], axis=1)                     # [N, 65]
    svp = np.ascontiguousarray(
        svaug.reshape(N // NCH, NCH, DA).transpose(1, 0, 2)
        .reshape(NCH, (N // NCH) * DA)).astype(_BF16)              # [128, 64*65]
    ident = np.eye(128, dtype=np.float32)
    wt = np.concatenate(
        [RELU_COEF * fc_w.T.astype(np.float32),
         fc_b.astype(np.float32)[None, :]], axis=0)                # [65, OUT]
    rcol = np.zeros((DA, 1), np.float32)
    rcol[D, 0] = 1.0                    # R pass-through output column
    wt = np.concatenate([wt, rcol], axis=1).astype(_BF16)          # [65, OUT+1]
    epi = np.zeros((D, 8), np.float32)
    epi[:, 0:K] = field_centers.T
    epi[:, 4] = field_centers.mean(axis=0)
    adj = np.ascontiguousarray(adj.astype(np.float32))

    in_maps = []
    for c in range(N_CORES):
        in_maps.append({
            "adj": adj[c * M_SH:(c + 1) * M_SH],
            "svp": svp,
            "ident": ident,
            "wt": wt,
            "epi": epi,
        })
    return in_maps


def run(semantic_vec, adj, field_centers, fc_w, fc_b, trace=False, **kw):
    from concourse.bass_utils import run_bass_kernel_spmd

    nc = _build_graph()
    in_maps = _prep_in_maps(semantic_vec, adj, field_centers, fc_w, fc_b)
    res = run_bass_kernel_spmd(nc, in_maps, core_ids=list(range(N_CORES)),
                               trace=trace, **kw)
    out = np.concatenate([res.results[i]["out"] for i in range(N_CORES)],
                         axis=0)
    return out, res


def kernel(semantic_vec, adj, field_centers, fc_w, fc_b):
    out, _ = run(semantic_vec, adj, field_centers, fc_w, fc_b, trace=False)
    return out


# revision 20
# speedup vs baseline: 1.1562x; 1.1562x over previous
"""Trainium2 Bass kernel for nn_AdjEnsemble (gnn_message_passing).

Math: softmax rows of adj sum to 1, so adj_norm @ (sv - c_k) = adj_norm@sv - c_k.
With E = exp(-adj) (no max-subtraction needed: adj ~ N(0,1)) and
R = rowsum(E), the whole module collapses to

    t        = (E @ sv) / R                      # [N, D]
    features = mean_k lrelu(t - c_k)
             = 0.2475 * sum_k relu(t - c_k) + 0.01*(t - mean_k c_k)
    out      = relu(features @ fc_w.T + fc_b)    # [N, OUT]

Sharding: adj rows split across 8 cores ([1024, 8192] each); everything
else replicated. No collectives: each core's output rows stay local.

Per-core dataflow: DMA adj tiles (natural layout, f32) -> PE transpose
128x128 blocks into PSUM -> ACT exp(-x) PSUM->SBUF (bf16) -> PE matmul
accumulate S^T[65, m] = svaug^T @ E^T over n-chunks (svaug has a ones
column so row 64 of S^T is the softmax denominator R) -> small epilogue
on DVE -> fc matmul (bias folded in as an extra contraction row) -> relu
-> DMA out.
"""

import numpy as np
import ml_dtypes

_BF16 = ml_dtypes.bfloat16

N, D, K, OUT = 8192, 64, 4, 256
N_CORES = 8
M_SH = N // N_CORES          # 1024 adj rows per core
DA = D + 1                   # 65: sv columns + ones column
MSUP = 512                   # max m rows accumulated per PSUM accumulator
SUPS = [128, 512, 256, 128]  # uneven m_supers: small head (fast start) + small tail
NBLK = 2048                  # n columns per DMA'd adj tile
NCH = 128                    # n contraction chunk (PE partition dim)
LRELU_SLOPE = 0.01
RELU_COEF = (1.0 - LRELU_SLOPE) / 4.0       # 0.2475
G = LRELU_SLOPE / RELU_COEF                 # linear-term coefficient inside feat'

_GRAPH_CACHE = {}


def _build_graph():
    if "nc" in _GRAPH_CACHE:
        return _GRAPH_CACHE["nc"]

    import concourse.tile as tile
    from concourse import bacc, mybir

    f32 = mybir.dt.float32
    f32r = mybir.dt.float32r
    bf16 = mybir.dt.bfloat16
    Act = mybir.ActivationFunctionType
    Alu = mybir.AluOpType

    nc = bacc.Bacc("TRN2", target_bir_lowering=False, debug=False,
                   num_devices=N_CORES)

    adj_ext = nc.declare_dram_parameter("adj", [M_SH, N], f32r, isOutput=False)
    svp_ext = nc.declare_dram_parameter("svp", [128, (N // NCH) * DA], bf16,
                                        isOutput=False)
    id_ext = nc.declare_dram_parameter("ident", [128, 128], f32r, isOutput=False)
    wt_ext = nc.declare_dram_parameter("wt", [DA, OUT + 1], bf16, isOutput=False)
    epi_ext = nc.declare_dram_parameter("epi", [D, 8], f32, isOutput=False)
    out_ext = nc.declare_dram_parameter("out", [M_SH, OUT], f32, isOutput=True)

    n_msup = M_SH // MSUP            # 2
    n_nblk = N // NBLK               # 4
    n_nch = NBLK // NCH              # 16
    n_sub = MSUP // 128              # 4

    with tile.TileContext(nc) as tc:
        with (
            tc.tile_pool(name="const", bufs=1) as const,
            tc.tile_pool(name="a", bufs=2 * n_sub) as apool,
            tc.tile_pool(name="et", bufs=3) as etpool,
            tc.tile_pool(name="epi", bufs=2) as epool,
            tc.tile_pool(name="osb", bufs=2) as outp,
            tc.tile_pool(name="stage", bufs=2, space="PSUM") as stpool,
            tc.tile_pool(name="acc", bufs=2, space="PSUM") as accpool,
            tc.tile_pool(name="ops", bufs=2, space="PSUM") as opool,
        ):
            svt = const.tile([128, (N // NCH) * DA], bf16)
            nc.sync.dma_start(svt[:], svp_ext[:])
            idt = const.tile([128, 128], f32r)
            nc.sync.dma_start(idt[:], id_ext[:])
            wtt = const.tile([DA, OUT + 1], bf16)
            nc.sync.dma_start(wtt[:], wt_ext[:])
            epit = const.tile([D, 8], f32)
            nc.sync.dma_start(epit[:], epi_ext[:])
            onest = const.tile([1, D], f32)
            nc.vector.memset(onest[:], 1.0)

            def emit_main(su, acc, nb_range, dma_eng=None):
                m0 = sum(SUPS[:su])
                w = SUPS[su]
                nsub = w // 128
                for nb in nb_range:
                    ats = []
                    for s in range(nsub):
                        at = apool.tile([128, NBLK], f32r, tag="a")
                        eng = dma_eng or nc.gpsimd
                        eng.dma_start(
                            at[:],
                            adj_ext[m0 + s * 128:m0 + (s + 1) * 128,
                                    nb * NBLK:(nb + 1) * NBLK])
                        ats.append(at)
                    for nch2 in range(n_nch // 2):
                        stage = stpool.tile([128, 2 * w], f32r, tag="stage")
                        for j in range(2):
                            nch = nch2 * 2 + j
                            for s in range(nsub):
                                nc.tensor.transpose(
                                    stage[:, j * w + s * 128:
                                          j * w + (s + 1) * 128],
                                    ats[s][:, nch * NCH:(nch + 1) * NCH],
                                    idt[:])
                        et = etpool.tile([128, 2 * w], bf16, tag="et")
                        nc.scalar.activation(et[:], stage[:], Act.Exp,
                                             scale=-1.0)
                        for j in range(2):
                            chunk = nb * n_nch + nch2 * 2 + j
                            nc.tensor.matmul(
                                acc[:, 0:w],
                                svt[:, chunk * DA:(chunk + 1) * DA],
                                et[:, j * w:(j + 1) * w],
                                start=(chunk == 0),
                                stop=(chunk == (N // NCH) - 1))

            def emit_epilogue(su, acc):
                # acc[0:64] = S^T rows, acc[64] = R (softmax denominator).
                # Division by R is deferred: G = sum_k relu(S - R c_k)
                # + g (S - R cbar); the fc matmul passes R through an extra
                # output column, and the final relu+divide happens in natural
                # layout where 1/R is a per-partition scalar.
                m0 = sum(SUPS[:su])
                w = SUPS[su]
                rsb = epool.tile([1, MSUP], f32, tag="rsb", name=f"rsb{su}")
                nc.vector.tensor_copy(rsb[0:1, 0:w], acc[D:DA, 0:w])
                rb_ps = opool.tile([D, MSUP], f32, tag="ops", name=f"rbp{su}")
                nc.tensor.matmul(rb_ps[:, 0:w], onest[:], rsb[0:1, 0:w],
                                 start=True, stop=True)
                rbs = epool.tile([D, MSUP], f32, tag="rbs", name=f"rbs{su}")
                nc.vector.tensor_copy(rbs[:, 0:w], rb_ps[:, 0:w])

                rk = []
                for k in range(K):
                    v = epool.tile([D, MSUP], f32, tag=f"v{k}",
                                   name=f"v{k}_{su}")
                    nc.vector.tensor_scalar_mul(v[:, 0:w], rbs[:, 0:w],
                                                epit[:, k:k + 1])
                    u = epool.tile([D, MSUP], f32, tag=f"u{k}",
                                   name=f"u{k}_{su}")
                    nc.vector.tensor_tensor(u[:, 0:w], acc[0:D, 0:w],
                                            v[:, 0:w], Alu.subtract)
                    r = epool.tile([D, MSUP], f32, tag=f"r{k}",
                                   name=f"r{k}_{su}")
                    nc.scalar.activation(r[:, 0:w], u[:, 0:w], Act.Relu)
                    rk.append(r)
                vb = epool.tile([D, MSUP], f32, tag="vb", name=f"vb{su}")
                nc.vector.tensor_scalar_mul(vb[:, 0:w], rbs[:, 0:w],
                                            epit[:, 4:5])
                qs = epool.tile([D, MSUP], f32, tag="qs", name=f"qs{su}")
                nc.vector.tensor_tensor(qs[:, 0:w], acc[0:D, 0:w],
                                        vb[:, 0:w], Alu.subtract)
                s01 = epool.tile([D, MSUP], f32, tag="s01", name=f"s01_{su}")
                nc.vector.tensor_tensor(s01[:, 0:w], rk[0][:, 0:w],
                                        rk[1][:, 0:w], Alu.add)
                s23 = epool.tile([D, MSUP], f32, tag="s23", name=f"s23_{su}")
                nc.vector.tensor_tensor(s23[:, 0:w], rk[2][:, 0:w],
                                        rk[3][:, 0:w], Alu.add)
                s03 = epool.tile([D, MSUP], f32, tag="s03", name=f"s03_{su}")
                nc.vector.tensor_tensor(s03[:, 0:w], s01[:, 0:w],
                                        s23[:, 0:w], Alu.add)
                qg = epool.tile([D, MSUP], f32, tag="qg", name=f"qg{su}")
                nc.vector.tensor_scalar(qg[:, 0:w], qs[:, 0:w], G, 0.0,
                                        Alu.mult, Alu.bypass)
                feat = epool.tile([DA, MSUP], bf16, tag="feat",
                                  name=f"feat{su}")
                nc.vector.tensor_copy(feat[D:DA, 0:w], rsb[0:1, 0:w])
                nc.vector.tensor_tensor(feat[0:D, 0:w], s03[:, 0:w],
                                        qg[:, 0:w], Alu.add)

                for mc in range(w // 128):
                    ops = opool.tile([128, OUT + 1], f32, tag="ops",
                                     name=f"ops{su}_{mc}")
                    nc.tensor.matmul(ops[:],
                                     feat[:, mc * 128:(mc + 1) * 128],
                                     wtt[:], start=True, stop=True)
                    rinv = outp.tile([128, 1], f32, tag="rinv",
                                     name=f"rinv{su}_{mc}")
                    nc.vector.reciprocal(rinv[:], ops[:, OUT:OUT + 1])
                    osb = outp.tile([128, OUT], f32, tag="osb",
                                    name=f"osb{su}_{mc}")
                    nc.vector.tensor_scalar(osb[:], ops[:, 0:OUT], 0.0,
                                            rinv[:], Alu.max, Alu.mult)
                    nc.sync.dma_start(
                        out_ext[m0 + mc * 128:m0 + (mc + 1) * 128, :],
                        osb[:])

            # PE warmup: dummy matmuls on the identity engage the HAM
            # clock-gate and bridge the initial DMA fill.
            warm = stpool.tile([128, 128], f32, tag="stage", name="warm")
            for _ in range(30):
                nc.tensor.matmul(warm[:], idt[:], idt[:],
                                 start=True, stop=True)

            accs = [accpool.tile([DA, MSUP], f32, tag="acc", name=f"acc{i}")
                    for i in range(len(SUPS))]
            emit_main(0, accs[0], range(n_nblk))
            emit_main(1, accs[1], range(0, 2))
            emit_epilogue(0, accs[0])
            emit_main(1, accs[1], range(2, n_nblk))
            emit_main(2, accs[2], range(0, 1))
            emit_epilogue(1, accs[1])
            emit_main(2, accs[2], range(1, n_nblk))
            emit_main(3, accs[3], range(0, 2))
            emit_epilogue(2, accs[2])
            emit_main(3, accs[3], range(2, n_nblk))
            emit_epilogue(3, accs[3])

    nc.compile()
    _GRAPH_CACHE["nc"] = nc
    return nc


def _prep_in_maps(semantic_vec, adj, field_centers, fc_w, fc_b):
    svaug = np.concatenate(
        [semantic_vec.astype(np.float32),
         np.ones((N, 1), np.float32)], axis=1)                     # [N, 65]
    svp = np.ascontiguousarray(
        svaug.reshape(N // NCH, NCH, DA).transpose(1, 0, 2)
        .reshape(NCH, (N // NCH) * DA)).astype(_BF16)              # [128, 64*65]
    ident = np.eye(128, dtype=np.float32)
    wt = np.concatenate(
        [RELU_COEF * fc_w.T.astype(np.float32),
         fc_b.astype(np.float32)[None, :]], axis=0)                # [65, OUT]
    rcol = np.zeros((DA, 1), np.float32)
    rcol[D, 0] = 1.0                    # R pass-through output column
    wt = np.concatenate([wt, rcol], axis=1).astype(_BF16)          # [65, OUT+1]
    epi = np.zeros((D, 8), np.float32)
    epi[:, 0:K] = field_centers.T
    epi[:, 4] = field_centers.mean(axis=0)
    adj = np.ascontiguousarray(adj.astype(np.float32))

    in_maps = []
    for c in range(N_CORES):
        in_maps.append({
            "adj": adj[c * M_SH:(c + 1) * M_SH],
            "svp": svp,
            "ident": ident,
            "wt": wt,
            "epi": epi,
        })
    return in_maps


def run(semantic_vec, adj, field_centers, fc_w, fc_b, trace=False, **kw):
    from concourse.bass_utils import run_bass_kernel_spmd

    nc = _build_graph()
    in_maps = _prep_in_maps(semantic_vec, adj, field_centers, fc_w, fc_b)
    res = run_bass_kernel_spmd(nc, in_maps, core_ids=list(range(N_CORES)),
                               trace=trace, **kw)
    out = np.concatenate([res.results[i]["out"] for i in range(N_CORES)],
                         axis=0)
    return out, res


def kernel(semantic_vec, adj, field_centers, fc_w, fc_b):
    out, _ = run(semantic_vec, adj, field_centers, fc_w, fc_b, trace=False)
    return out
